# revision 21
# baseline (speedup 1.0000x reference)
"""Trainium2 Bass kernel for nn_BaseModel_14499809591724 (GNN message passing).

Strategy (8 NeuronCores, data-parallel over graph batches):
  - Nodes split into 8 contiguous shards at graph boundaries (batch sorted),
    padded to S=6400 rows; replicated node table = [8*S, 128] bf16, stored
    chunk-major in 2 chunks of 25600 rows (int16 gather-index safe).
  - Self-loops are folded into the edge list as explicit (n -> n) edges with
    weight 1/deg, so a conv is purely gather + one-hot scatter matmul.
  - Per GCN conv, per window-pair (2 x 128 dst nodes) and table chunk: one
    dma_gather pulls the source rows (round-robin over 4 SWDGE queues so Q7
    descriptor generation runs 4-wide); the matching scaled one-hot tiles
    (host-precomputed, bf16) stream in from DRAM; PE contracts
    aggT[f,dst] += G[e,f]^T @ OH[e,dst] in PSUM.  Chunk-0 partials are
    evicted to SBUF by ACT and re-injected via an identity matmul in the
    chunk-1 pass.  W/bias/ReLU apply feature-major with no transposes; only
    convs feeding an AllGather transpose back to node-major.
  - The vector engine is kept COLD during convs: its SBUF port contends with
    GPSIMD's descriptor generation and halves gather throughput.
  - After each conv that feeds another conv, the 8 shards are AllGathered
    (bf16, 2 chunks, overlapped with compute).
  - JumpingKnowledge + per-graph pooling (one-hot matmul) + BN + MLP head +
    softmax run per core on its own 64 graphs; host concatenates 8 x [64,10].
"""
import sys
import numpy as np
import ml_dtypes

sys.path.insert(0, "/opt/trn_rl_repo")

from concourse import bacc, tile, mybir  # noqa: E402
from concourse.bass_utils import run_bass_kernel_spmd  # noqa: E402

# ---- model / sharding constants (shapes fixed by the problem) ----
NC = 8
N_NODES = 50000
N_EDGES = 800000
F = 128
B = 512
GPC = B // NC          # graphs per core = 64
S = 6400               # padded nodes per shard (max real shard is 6368)
NW = S // 128          # 50 windows per core
NWP = NW // 2          # 25 window pairs
TAB = NC * S           # 51200 table rows
NCH = 2                # table chunks (progressive AllGather pipeline)
CHS = S // NCH         # 3200 shard rows per chunk
CHROWS = NC * CHS      # 25600 table rows per chunk (int16-safe)
NB = 3
BN_EPS = 1e-5
NQ = 4                 # SWDGE queues for gather descriptor generation
OHSCALE = 128.0        # one-hot fp8 pre-scale; 1/OHSCALE folded into conv_w

f32 = mybir.dt.float32
bf16 = mybir.dt.bfloat16
i16 = mybir.dt.int16
fp8 = mybir.dt.float8e4


def _ch_of(w: int, k: int) -> int:
    """Groups for bucket (window w, chunk k). The chunk holding w's own nodes
    also receives w's 128 self-edges, so it gets one extra group."""
    return 10 if k == (0 if w < NW // 2 else 1) else 9


# static slot/group layout: units are (k, wp) in k-major order; each unit is
# one gather covering buckets (2wp, k) then (2wp+1, k).
UNITS = [(k, wp) for k in range(NCH) for wp in range(NWP)]
UNIT_CH = {(k, wp): (_ch_of(2 * wp, k), _ch_of(2 * wp + 1, k)) for k, wp in UNITS}
UNIT_NIDX = {u: (UNIT_CH[u][0] + UNIT_CH[u][1]) * 128 for u in UNITS}
UNIT_SLOT = {}
UNIT_GBASE = {}
UNIT_GCOL = {}
_s = _g = _c = 0
for _u in UNITS:
    UNIT_SLOT[_u] = _s
    UNIT_GBASE[_u] = _g
    UNIT_GCOL[_u] = _c
    _s += UNIT_NIDX[_u]
    _g += UNIT_NIDX[_u] // 128
    _c += UNIT_NIDX[_u] // 16
NSLOTS = _s            # 121600
NGROUPS = _g           # 950
GIDXCOLS = _c          # 7600
MAXG = max(UNIT_NIDX[u] // 128 for u in UNITS)  # 20

NCU = 18               # OH units cached persistently in SBUF
CACHED_UNITS = [u for i, u in enumerate(UNITS) if i % 2 == 0][:NCU]
_cs = set(CACHED_UNITS)
UNCACHED_UNITS = [u for u in UNITS if u not in _cs]
# ohcache column layout: cached units' groups first, then uncached
OHGB = {}
_g2 = 0
for _u in CACHED_UNITS + UNCACHED_UNITS:
    OHGB[_u] = _g2
    _g2 += UNIT_NIDX[_u] // 128
CGROUPS = sum(UNIT_NIDX[u] // 128 for u in CACHED_UNITS)

_PROGRAM = None
# tuned pipeline depths (see session notes: gather/one-hot prefetch depth
# dominates; PSUM is bank-granular so pp+hn+hnT+pooled must total <= 8)
SKIP_OH = SKIP_FIN = SKIP_PART = SKIP_G = False
GB = 8      # gather destination buffers (deep SWDGE pipeline)
OHB = 12    # one-hot load buffers (deep HWDGE prefetch)
PPB = 4     # PSUM accumulation banks
OH_ENG = "sync"
OHPRI = GPRI = 0
GQN = 4     # SWDGE queues used round-robin by gathers
AG_DECOUPLE = False
SP1 = False
AGMODE = 2
REPEAT = 1


def _preprocess(inp: dict):
    batch = np.asarray(inp["batch"])
    ei = np.asarray(inp["edge_index"])
    ew = np.asarray(inp["edge_attr"], dtype=np.float32)
    x = np.asarray(inp["x"], dtype=np.float32)
    src, dst = ei[0].astype(np.int64), ei[1].astype(np.int64)

    bounds = np.searchsorted(batch, np.arange(0, B + 1, GPC)).astype(np.int64)
    sizes = np.diff(bounds)
    assert sizes.max() <= S, f"shard overflow: {sizes.max()} > {S}"

    node = np.arange(N_NODES, dtype=np.int64)
    core_of = (np.searchsorted(bounds, node, side="right") - 1).astype(np.int64)
    off = node - bounds[core_of]
    # chunk-major table: row = chunk*CHROWS + core*CHS + (off % CHS)
    tab = (off // CHS) * CHROWS + core_of * CHS + (off % CHS)

    deg = (np.bincount(dst, weights=ew.astype(np.float64), minlength=N_NODES) + 1.0)
    deg = deg.astype(np.float32)
    dinv = 1.0 / np.sqrt(deg)
    norm = (dinv[src] * ew * dinv[dst]).astype(np.float32)
    dinv2 = (1.0 / deg).astype(np.float32)

    # append self-edges: src=dst=n with weight 1/deg(n)
    src_a = np.concatenate([src, node])
    dst_a = np.concatenate([dst, node])
    norm_a = np.concatenate([norm, dinv2])

    # full replicated x table (node-major, bf16)
    xtab = np.zeros((TAB, F), dtype=ml_dtypes.bfloat16)
    xtab[tab] = x.astype(ml_dtypes.bfloat16)

    identf = np.eye(128, dtype=np.float32)
    identb = np.eye(128, dtype=ml_dtypes.bfloat16)

    # weights
    conv_w = np.asarray(inp["conv_w"], dtype=np.float32).reshape(6, F, F)
    convw = (conv_w.transpose(1, 0, 2).reshape(F, 6 * F)
             / OHSCALE).astype(ml_dtypes.bfloat16)
    convb = np.asarray(inp["conv_b"], dtype=np.float32).reshape(6, F).T.copy()
    jk_w = np.asarray(inp["jk_w"], dtype=np.float32).reshape(NB, 2, F, F).reshape(6, F, F)
    jkw = jk_w.transpose(1, 0, 2).reshape(F, 6 * F).astype(ml_dtypes.bfloat16)
    jkb = np.asarray(inp["jk_b"], dtype=np.float32).T.copy()
    sc = (np.asarray(inp["bn_gamma"], dtype=np.float32)
          / np.sqrt(np.asarray(inp["bn_var"], dtype=np.float32) + BN_EPS))
    tr = (np.asarray(inp["bn_beta"], dtype=np.float32)
          - np.asarray(inp["bn_mean"], dtype=np.float32) * sc)
    bns = sc.reshape(NB, F).T.copy()
    bnt = tr.reshape(NB, F).T.copy()
    lin1_w = np.asarray(inp["lin1_w"], dtype=np.float32).reshape(NB, F, F)
    l1w = lin1_w.transpose(1, 0, 2).reshape(F, NB * F).copy()
    l1b = np.asarray(inp["lin1_b"], dtype=np.float32).reshape(F, 1).copy()
    l2w = np.asarray(inp["lin2_w"], dtype=np.float32).copy()
    l2b = np.asarray(inp["lin2_b"], dtype=np.float32).reshape(10, 1).copy()

    shared = {
        "xtab": xtab, "identf": identf, "identb": identb,
        "convw": convw, "convb": convb, "jkw": jkw, "jkb": jkb,
        "bns": bns, "bnt": bnt, "l1w": l1w, "l1b": l1b, "l2w": l2w, "l2b": l2b,
    }

    dst_core = core_of[dst_a]
    dst_off = off[dst_a]
    src_tab = tab[src_a]

    # per-bucket slot bases in the unit-major layout
    bucket_base = np.empty(NW * NCH, dtype=np.int64)  # bid = w*NCH + k
    bucket_cap = np.empty(NW * NCH, dtype=np.int64)
    for k, wp in UNITS:
        cha, chb = UNIT_CH[(k, wp)]
        ub = UNIT_SLOT[(k, wp)]
        bucket_base[(2 * wp) * NCH + k] = ub
        bucket_base[(2 * wp + 1) * NCH + k] = ub + cha * 128
        bucket_cap[(2 * wp) * NCH + k] = cha * 128
        bucket_cap[(2 * wp + 1) * NCH + k] = chb * 128

    in_maps = []
    for c in range(NC):
        eidx = np.flatnonzero(dst_core == c)
        e_w = dst_off[eidx] // 128
        e_k = src_tab[eidx] // CHROWS
        bid = e_w * NCH + e_k
        order = np.argsort(bid, kind="stable")
        eidx = eidx[order]
        bid = bid[order]
        counts = np.bincount(bid, minlength=NW * NCH)
        assert (counts <= bucket_cap).all(), (
            f"bucket overflow core {c}: {counts.max()}")
        starts = np.concatenate([[0], np.cumsum(counts)])[:-1]
        pos = np.arange(len(eidx)) - starts[bid]
        slot = bucket_base[bid] + pos

        idx_slots = np.zeros(NSLOTS, dtype=np.int64)
        idx_slots[slot] = src_tab[eidx] % CHROWS
        rel_slots = np.zeros(NSLOTS, dtype=np.int64)
        rel_slots[slot] = dst_off[eidx] % 128
        nrm_slots = np.zeros(NSLOTS, dtype=np.float32)
        nrm_slots[slot] = norm_a[eidx]

        # wrapped gather idx: per unit [16, nidx/16], concatenated, tiled x8
        gcols = []
        for u in UNITS:
            n = UNIT_NIDX[u]
            runs = idx_slots[UNIT_SLOT[u]:UNIT_SLOT[u] + n]
            gcols.append(runs.reshape(n // 16, 16).T)
        gidx = np.tile(np.concatenate(gcols, axis=1).astype(np.int16), (8, 1))

        # host-built scaled one-hot tiles: [128, NGROUPS*128] fp8, with the
        # cached units' groups packed first (SBUF-resident across convs)
        oh = np.zeros((NGROUPS, 128, 128), dtype=np.float32)
        grp = slot // 128
        oh[grp, slot % 128, rel_slots[slot]] = nrm_slots[slot] * OHSCALE
        perm = np.empty(NGROUPS, dtype=np.int64)
        for u in UNITS:
            n = UNIT_NIDX[u] // 128
            perm[OHGB[u]:OHGB[u] + n] = np.arange(UNIT_GBASE[u],
                                                  UNIT_GBASE[u] + n)
        ohc = np.ascontiguousarray(oh[perm].transpose(1, 0, 2)).reshape(
            128, NGROUPS * 128).astype(ml_dtypes.float8_e4m3)

        # per-graph pooling one-hot
        ln = np.arange(sizes[c], dtype=np.int64)
        pool = np.zeros((128, NW * GPC), dtype=ml_dtypes.bfloat16)
        g_of = batch[bounds[c] + ln].astype(np.int64) - c * GPC
        pool[ln % 128, (ln // 128) * GPC + g_of] = 1.0

        m = {"gidx": gidx, "ohcache": ohc, "pool": pool}
        m.update(shared)
        in_maps.append(m)
    return in_maps


def _build_program(stage=99):
    nc = bacc.Bacc("TRN2", target_bir_lowering=False, debug=False,
                   num_devices=NC, num_swdge_queues=NQ)
    AF = mybir.ActivationFunctionType
    OP = mybir.AluOpType

    ap = {}
    big_inputs = ([] if stage == 0 else [
        ("xtab", [TAB, F], bf16),
        ("gidx", [128, GIDXCOLS], i16),
        ("ohcache", [128, NGROUPS * 128], fp8),
        ("pool", [128, NW * GPC], bf16),
    ])
    for name, shape, dt in big_inputs + [
        ("identf", [128, 128], f32), ("identb", [128, 128], bf16),
        ("convw", [F, 6 * F], bf16), ("convb", [F, 6], f32),
        ("jkw", [F, 6 * F], bf16), ("jkb", [F, NB], f32),
        ("bns", [F, NB], f32), ("bnt", [F, NB], f32),
        ("l1w", [F, NB * F], f32), ("l1b", [F, 1], f32),
        ("l2w", [F, 10], f32), ("l2b", [10, 1], f32),
    ]:
        ap[name] = nc.dram_tensor(name, shape, dt, kind="ExternalInput").ap()
    out_ap = nc.dram_tensor("out", [GPC, 10], f32, kind="ExternalOutput").ap()

    with tile.TileContext(nc) as tc:
        with (
            tc.tile_pool(name="dram", bufs=1, space="DRAM") as dram,
            tc.tile_pool(name="pers", bufs=1) as pers,
            tc.tile_pool(name="rot", bufs=1) as rot,
            tc.tile_pool(name="psum", bufs=1, space="PSUM") as psum,
        ):
            ag_in = dram.tile([S, F], bf16)
            ag_const = dram.tile([S, F], bf16)

            # ---- persistent SBUF loads
            sb = {}
            for name in ((["gidx", "pool"] if stage > 0 else []) +
                         ["identf", "identb", "convw", "convb",
                          "jkw", "jkb", "bns", "bnt", "l1w", "l1b", "l2w",
                          "l2b"]):
                t_ = pers.tile(list(ap[name].shape), ap[name].dtype,
                               name=f"sb_{name}")
                nc.sync.dma_start(t_[:], ap[name][:])
                sb[name] = t_

            h1_fm = pers.tile([128, S], bf16, name="h1_fm")
            if SKIP_FIN:
                nc.scalar.copy(h1_fm[:, 0:128], sb["identb"][:])
            h2_fm = pers.tile([128, S], bf16, name="h2_fm")
            hb_fm = pers.tile([128, S], bf16, name="hb_fm")
            part = pers.tile([128, NW, 128], bf16, name="part")
            z_sb = pers.tile([128, NB, GPC], f32, name="z_sb")

            qctr = [0]
            _cset = set(CACHED_UNITS)
            if stage == 0:
                ohsb = None
            if AG_DECOUPLE:
                for _w in range(NW):
                    nc.sync.dma_start(ag_const[_w * 128:(_w + 1) * 128, :],
                                      sb["identb"][:])
            if stage > 0:
                Gpre = pers.tile([128, MAXG, F], bf16, name="Gpre")
                nc.sync.dma_start(Gpre[:, 0:1, :], ap["xtab"][0:128, :])
                OHpre = pers.tile([128, MAXG, 128], fp8, name="OHpre")
                nc.sync.dma_start(OHpre[:, 0:1, :], ap["ohcache"][:, 0:128])
            if CGROUPS and stage > 0:
                ohsb = pers.tile([128, CGROUPS, 128], fp8, name="ohsb")
                nc.sync.dma_start(ohsb[:], ap["ohcache"][:, 0:CGROUPS * 128])
            else:
                ohsb = None

            def unit_tiles(tables, k, wp):
                u = (k, wp)
                ng = UNIT_NIDX[u] // 128
                if SKIP_G:
                    G = Gpre
                else:
                    from contextlib import nullcontext
                    G = rot.tile([128, MAXG, F], bf16, tag="G", bufs=GB, name="G")
                    with (tc.high_priority(offset=GPRI) if GPRI
                          else nullcontext()):
                        nc.gpsimd.dma_gather(
                            out_ap=G[:, 0:ng, :], in_ap=tables[k][:],
                            idxs_ap=sb["gidx"][:, UNIT_GCOL[u]:
                                               UNIT_GCOL[u] + UNIT_NIDX[u] // 16],
                            num_idxs=UNIT_NIDX[u], num_idxs_reg=UNIT_NIDX[u],
                            elem_size=F, single_packet=SP1,
                            queue_num=qctr[0] % GQN)
                    qctr[0] += 1
                if SKIP_OH:
                    OH, ob = OHpre, 0
                elif u in _cset:
                    OH, ob = ohsb, OHGB[u]
                else:
                    from contextlib import nullcontext
                    OH = rot.tile([128, MAXG, 128], fp8, tag="OH", bufs=OHB,
                                  name="OH")
                    src_ap = ap["ohcache"][:, OHGB[u] * 128:
                                           (OHGB[u] + ng) * 128]
                    with (tc.high_priority(offset=OHPRI) if OHPRI
                          else nullcontext()):
                        if OH_ENG == "gps3":
                            nc.gpsimd.dma_start(OH[:, 0:ng, :], src_ap,
                                                queue_num=NQ - 1)
                        elif OH_ENG == "act":
                            nc.scalar.dma_start(OH[:, 0:ng, :], src_ap)
                        else:
                            nc.sync.dma_start(OH[:, 0:ng, :], src_ap)
                    ob = 0
                return G, OH, ob

            def ag_chunk(i, k):
                src = ag_const if AG_DECOUPLE else ag_in
                tk = dram.tile([CHROWS, F], bf16, addr_space="Shared",
                               tag=f"t{_rep[0]}_{i}_{k}",
                               name=f"t{_rep[0]}_{i}_{k}")
                nc.gpsimd.collective_compute(
                    "AllGather", OP.bypass,
                    replica_groups=[list(range(NC))],
                    ins=[src[k * CHS:(k + 1) * CHS, :].opt()],
                    outs=[tk.opt()])
                # gathers from Shared-space DRAM run ~60% slower than from
                # regular DRAM; copy the table out before gathering from it
                tl = dram.tile([CHROWS, F], bf16,
                               tag=f"l{_rep[0]}_{i}_{k}",
                               name=f"l{_rep[0]}_{i}_{k}")
                nc.sync.dma_start(tl[:], tk[:])
                return tl

            def ag_full(i):
                # single collective for the whole shard (core-major output),
                # reshuffled to the 2 chunk-major tables during the copy-out
                src = ag_const if AG_DECOUPLE else ag_in
                tk = dram.tile([NC * S, F], bf16, addr_space="Shared",
                               tag=f"t{_rep[0]}_{i}", name=f"t{_rep[0]}_{i}")
                nc.gpsimd.collective_compute(
                    "AllGather", OP.bypass,
                    replica_groups=[list(range(NC))],
                    ins=[src[:].opt()], outs=[tk.opt()])
                tabs = []
                for k in range(NCH):
                    tl = dram.tile([CHROWS, F], bf16,
                                   tag=f"l{_rep[0]}_{i}_{k}",
                                   name=f"l{_rep[0]}_{i}_{k}")
                    for co in range(NC):
                        eng = nc.sync if co % 2 == 0 else nc.scalar
                        eng.dma_start(
                            tl[co * CHS:(co + 1) * CHS, :],
                            tk[co * S + k * CHS:co * S + (k + 1) * CHS, :])
                    tabs.append(tl)
                return tabs

            def conv(lk, tables, h_out, ag, jkli=None):
                if jkli is not None:
                    pooled = psum.tile([128, GPC], f32, tag="pooled", bufs=1,
                                       name="pooled")
                # chunk-0 pass: accumulate in PSUM, evict partials to SBUF
                for wp in range(NWP):
                    G, OH, ob = unit_tiles(tables, 0, wp)
                    cha, chb = UNIT_CH[(0, wp)]
                    for half, ch in ((0, cha), (1, chb)):
                        w = 2 * wp + half
                        gb = half * cha
                        pp = psum.tile([128, 128], f32, tag="pp", bufs=PPB,
                                       name="pp")
                        for c in range(ch):
                            nc.tensor.matmul(pp[:], G[:, gb + c, :],
                                             OH[:, ob + gb + c, :],
                                             start=(c == 0), stop=(c == ch - 1))
                        if not SKIP_PART:
                            nc.scalar.copy(part[:, w, :], pp[:])
                # chunk-1 pass: re-inject partial, finish, apply W+bias+ReLU
                tabs = []
                for wp in range(NWP):
                    G, OH, ob = unit_tiles(tables, 1, wp)
                    cha, chb = UNIT_CH[(1, wp)]
                    for half, ch in ((0, cha), (1, chb)):
                        w = 2 * wp + half
                        gb = half * cha
                        pp = psum.tile([128, 128], f32, tag="pp", bufs=PPB,
                                       name="pp")
                        if SKIP_PART:
                            nc.tensor.matmul(pp[:], sb["identb"][:],
                                             sb["identb"][:],
                                             start=True, stop=False)
                        else:
                            nc.tensor.matmul(pp[:], sb["identb"][:],
                                             part[:, w, :],
                                             start=True, stop=False)
                        for c in range(ch):
                            nc.tensor.matmul(pp[:], G[:, gb + c, :],
                                             OH[:, ob + gb + c, :],
                                             start=False, stop=(c == ch - 1))
                        if SKIP_FIN:
                            continue
                        aggs = rot.tile([128, 128], bf16, tag="aggs", bufs=4,
                                        name="aggs")
                        nc.scalar.copy(aggs[:], pp[:])
                        hn = psum.tile([128, 128], f32, tag="hn", bufs=2,
                                       name="hn")
                        nc.tensor.matmul(hn[:], sb["convw"][:, lk * F:(lk + 1) * F],
                                         aggs[:], start=True, stop=True)
                        nc.scalar.activation(h_out[:, w * 128:(w + 1) * 128],
                                             hn[:], AF.Relu,
                                             bias=sb["convb"][:, lk:lk + 1])
                        if jkli is not None:
                            # fused JumpingKnowledge: hb = relu(jk_w @ [h1;h2])
                            li = jkli
                            hb = psum.tile([128, 128], f32, tag="hn", bufs=2,
                                           name="hb")
                            nc.tensor.matmul(
                                hb[:],
                                sb["jkw"][:, (2 * li) * F:(2 * li + 1) * F],
                                h1_fm[:, w * 128:(w + 1) * 128],
                                start=True, stop=False)
                            nc.tensor.matmul(
                                hb[:],
                                sb["jkw"][:, (2 * li + 1) * F:(2 * li + 2) * F],
                                h_out[:, w * 128:(w + 1) * 128],
                                start=False, stop=True)
                            nc.scalar.activation(hb_fm[:, w * 128:(w + 1) * 128],
                                                 hb[:], AF.Relu,
                                                 bias=sb["jkb"][:, li:li + 1])
                            src_fm = hb_fm
                        else:
                            src_fm = h_out
                        if ag is not None or jkli is not None:
                            hnT = psum.tile([128, 128], bf16, tag="hnT",
                                            bufs=1, name="hnT")
                            nc.tensor.transpose(hnT[:],
                                                src_fm[:, w * 128:(w + 1) * 128],
                                                sb["identb"][:])
                            hcol = rot.tile([128, 128], bf16, tag="hcol",
                                            bufs=4, name="hcol")
                            nc.scalar.copy(hcol[:], hnT[:])
                        if jkli is not None:
                            nc.tensor.matmul(pooled[:], hcol[:],
                                             sb["pool"][:, w * GPC:(w + 1) * GPC],
                                             start=(w == 0), stop=(w == NW - 1))
                        if ag is not None:
                            nc.sync.dma_start(ag_in[w * 128:(w + 1) * 128, :],
                                              hcol[:])
                            # trigger the chunk AG as soon as its input rows
                            # are all written (w24 -> chunk0, w49 -> chunk1)
                            if AGMODE == 2:
                                if w == NW // 2 - 1:
                                    tabs.append(ag_chunk(ag, 0))
                                elif w == NW - 1:
                                    tabs.append(ag_chunk(ag, 1))
                            elif w == NW - 1:
                                tabs.extend(ag_full(ag))
                if jkli is not None:
                    nc.scalar.copy(z_sb[:, jkli, :], pooled[:])
                return tabs if ag is not None else None

            def jk(li, ag):
                last = ag is None
                tabs = []
                pooled = psum.tile([128, GPC], f32, tag="pooled", bufs=1,
                                   name="pooled")
                for w in range(NW):
                    hb = psum.tile([128, 128], f32, tag="hn", bufs=2,
                                   name="hb")
                    nc.tensor.matmul(hb[:],
                                     sb["jkw"][:, (2 * li) * F:(2 * li + 1) * F],
                                     h1_fm[:, w * 128:(w + 1) * 128],
                                     start=True, stop=False)
                    nc.tensor.matmul(hb[:],
                                     sb["jkw"][:, (2 * li + 1) * F:(2 * li + 2) * F],
                                     h2_fm[:, w * 128:(w + 1) * 128],
                                     start=False, stop=True)
                    nc.scalar.activation(hb_fm[:, w * 128:(w + 1) * 128],
                                         hb[:], AF.Relu,
                                         bias=sb["jkb"][:, li:li + 1])
                    hnT = psum.tile([128, 128], bf16, tag="hnT", bufs=1,
                                    name="hnT")
                    nc.tensor.transpose(hnT[:], hb_fm[:, w * 128:(w + 1) * 128],
                                        sb["identb"][:])
                    hcol = rot.tile([128, 128], bf16, tag="hcol", bufs=4,
                                    name="hcol")
                    nc.scalar.copy(hcol[:], hnT[:])
                    if not last:
                        nc.sync.dma_start(ag_in[w * 128:(w + 1) * 128, :],
                                          hcol[:])
                        if AGMODE == 2:
                            if w == NW // 2 - 1:
                                tabs.append(ag_chunk(ag, 0))
                            elif w == NW - 1:
                                tabs.append(ag_chunk(ag, 1))
                        elif w == NW - 1:
                            tabs.extend(ag_full(ag))
                    nc.tensor.matmul(pooled[:], hcol[:],
                                     sb["pool"][:, w * GPC:(w + 1) * GPC],
                                     start=(w == 0), stop=(w == NW - 1))
                nc.scalar.copy(z_sb[:, li, :], pooled[:])
                return tabs if not last else None

            # ---- main flow
            steps = [
                lambda: conv(0, [ap["xtab"][k * CHROWS:(k + 1) * CHROWS, :]
                                 for k in range(NCH)], h1_fm, 0),
                lambda t: conv(1, t, h2_fm, 1, jkli=0),
                lambda t: conv(2, t, h1_fm, 2),
                lambda t: conv(3, t, h2_fm, 3, jkli=1),
                lambda t: conv(4, t, h1_fm, 4),
                lambda t: conv(5, t, h2_fm, None, jkli=2),
            ]
            _rep = [0]
            for rep in range(REPEAT):
                _rep[0] = rep
                table = None
                for i, st in enumerate(steps):
                    if i >= stage:
                        break
                    r = st(table) if st.__code__.co_argcount else st()
                    if r is not None:
                        table = r

            # ---- head
            if stage < 6:
                outt0 = rot.tile([GPC, 10], f32, tag="outt", bufs=1,
                                 name="outt0")
                nc.vector.tensor_copy(outt0[:], h1_fm[0:GPC, 0:10])
                nc.sync.dma_start(out_ap[:], outt0[:])
            else:
                _head(nc, tc, rot, psum, sb, z_sb, out_ap)

    nc.compile()
    return nc


def _head(nc, tc, rot, psum, sb, z_sb, out_ap):
    AF = mybir.ActivationFunctionType
    OP = mybir.AluOpType
    zbn = rot.tile([128, NB, GPC], f32, tag="zbn", bufs=1, name="zbn")
    for t in range(NB):
        nc.vector.tensor_scalar(
            out=zbn[:, t, :], in0=z_sb[:, t, :],
            scalar1=sb["bns"][:, t:t + 1], scalar2=sb["bnt"][:, t:t + 1],
            op0=OP.mult, op1=OP.add)
    a1 = psum.tile([128, GPC], f32, tag="hn", bufs=2, name="a1")
    for t in range(NB):
        nc.tensor.matmul(a1[:], sb["l1w"][:, t * F:(t + 1) * F],
                         zbn[:, t, :], start=(t == 0), stop=(t == NB - 1))
    a1s = rot.tile([128, GPC], f32, tag="a1s", bufs=1, name="a1s")
    nc.scalar.activation(a1s[:], a1[:], AF.Relu, bias=sb["l1b"][:])
    z2 = psum.tile([10, GPC], f32, tag="pooled", bufs=1, name="z2")
    nc.tensor.matmul(z2[:], sb["l2w"][:], a1s[:], start=True, stop=True)
    z2s = rot.tile([10, GPC], f32, tag="z2s", bufs=1, name="z2s")
    nc.scalar.activation(z2s[:], z2[:], AF.Identity, bias=sb["l2b"][:])
    z2T = psum.tile([GPC, 10], f32, tag="hnT", bufs=1, name="z2T")
    nc.tensor.transpose(z2T[:], z2s[:], sb["identf"][0:10, 0:10])
    z2Ts = rot.tile([GPC, 10], f32, tag="z2Ts", bufs=1, name="z2Ts")
    nc.vector.tensor_copy(z2Ts[:], z2T[:])
    negm = rot.tile([GPC, 1], f32, tag="negm", bufs=1, name="negm")
    nc.vector.tensor_reduce(negm[:], z2Ts[:], mybir.AxisListType.X,
                            OP.max, negate=True)
    et = rot.tile([GPC, 10], f32, tag="et", bufs=1, name="et")
    nc.scalar.activation(et[:], z2Ts[:], AF.Exp, bias=negm[:])
    ssum = rot.tile([GPC, 1], f32, tag="ssum", bufs=1, name="ssum")
    nc.vector.tensor_reduce(ssum[:], et[:], mybir.AxisListType.X, OP.add)
    rcp = rot.tile([GPC, 1], f32, tag="rcp", bufs=1, name="rcp")
    nc.vector.reciprocal(rcp[:], ssum[:])
    outt = rot.tile([GPC, 10], f32, tag="outt", bufs=1, name="outt")
    nc.vector.tensor_scalar_mul(outt[:], et[:], rcp[:])
    nc.sync.dma_start(out_ap[:], outt[:])


def _get_program():
    global _PROGRAM
    if _PROGRAM is None:
        _PROGRAM = _build_program()
    return _PROGRAM


def kernel(**inputs) -> np.ndarray:
    in_maps = _preprocess(inputs)
    nc = _get_program()
    res = run_bass_kernel_spmd(nc, in_maps, list(range(NC)))
    return np.concatenate([res.results[c]["out"] for c in range(NC)], axis=0)


# revision 23
# speedup vs baseline: 1.0151x; 1.0151x over previous
"""Trainium2 Bass kernel for nn_BaseModel_14499809591724 (GNN message passing).

Strategy (8 NeuronCores, data-parallel over graph batches):
  - Nodes split into 8 contiguous shards at graph boundaries (batch sorted),
    padded to S=6400 rows; replicated node table = [8*S, 128] bf16, stored
    chunk-major in 2 chunks of 25600 rows (int16 gather-index safe).
  - Self-loops are folded into the edge list as explicit (n -> n) edges with
    weight 1/deg, so a conv is purely gather + one-hot scatter matmul.
  - Per GCN conv, per window-pair (2 x 128 dst nodes) and table chunk: one
    dma_gather pulls the source rows (round-robin over 4 SWDGE queues so Q7
    descriptor generation runs 4-wide); the matching scaled one-hot tiles
    (host-precomputed, bf16) stream in from DRAM; PE contracts
    aggT[f,dst] += G[e,f]^T @ OH[e,dst] in PSUM.  Chunk-0 partials are
    evicted to SBUF by ACT and re-injected via an identity matmul in the
    chunk-1 pass.  W/bias/ReLU apply feature-major with no transposes; only
    convs feeding an AllGather transpose back to node-major.
  - The vector engine is kept COLD during convs: its SBUF port contends with
    GPSIMD's descriptor generation and halves gather throughput.
  - After each conv that feeds another conv, the 8 shards are AllGathered
    (bf16, 2 chunks, overlapped with compute).
  - JumpingKnowledge + per-graph pooling (one-hot matmul) + BN + MLP head +
    softmax run per core on its own 64 graphs; host concatenates 8 x [64,10].
"""
import sys
import numpy as np
import ml_dtypes

sys.path.insert(0, "/opt/trn_rl_repo")

from concourse import bacc, tile, mybir  # noqa: E402
from concourse.bass_utils import run_bass_kernel_spmd  # noqa: E402

# ---- model / sharding constants (shapes fixed by the problem) ----
NC = 8
N_NODES = 50000
N_EDGES = 800000
F = 128
B = 512
GPC = B // NC          # graphs per core = 64
S = 6400               # padded nodes per shard (max real shard is 6368)
NW = S // 128          # 50 windows per core
NWP = NW // 2          # 25 window pairs
TAB = NC * S           # 51200 table rows
NCH = 2                # table chunks (progressive AllGather pipeline)
CHS = S // NCH         # 3200 shard rows per chunk
CHROWS = NC * CHS      # 25600 table rows per chunk (int16-safe)
NB = 3
BN_EPS = 1e-5
NQ = 4                 # SWDGE queues for gather descriptor generation
OHSCALE = 128.0        # one-hot fp8 pre-scale; 1/OHSCALE folded into conv_w

f32 = mybir.dt.float32
bf16 = mybir.dt.bfloat16
i16 = mybir.dt.int16
fp8 = mybir.dt.float8e4


def _ch_of(w: int, k: int) -> int:
    """Groups for bucket (window w, chunk k). The chunk holding w's own nodes
    also receives w's 128 self-edges, so it gets one extra group."""
    return 10 if k == (0 if w < NW // 2 else 1) else 9


# static slot/group layout: units are (k, wp) in k-major order; each unit is
# one gather covering buckets (2wp, k) then (2wp+1, k).
UNITS = [(k, wp) for k in range(NCH) for wp in range(NWP)]
UNIT_CH = {(k, wp): (_ch_of(2 * wp, k), _ch_of(2 * wp + 1, k)) for k, wp in UNITS}
UNIT_NIDX = {u: (UNIT_CH[u][0] + UNIT_CH[u][1]) * 128 for u in UNITS}
UNIT_SLOT = {}
UNIT_GBASE = {}
UNIT_GCOL = {}
_s = _g = _c = 0
for _u in UNITS:
    UNIT_SLOT[_u] = _s
    UNIT_GBASE[_u] = _g
    UNIT_GCOL[_u] = _c
    _s += UNIT_NIDX[_u]
    _g += UNIT_NIDX[_u] // 128
    _c += UNIT_NIDX[_u] // 16
NSLOTS = _s            # 121600
NGROUPS = _g           # 950
GIDXCOLS = _c          # 7600
MAXG = max(UNIT_NIDX[u] // 128 for u in UNITS)  # 20

NCU = 16               # OH units cached persistently in SBUF
CACHED_UNITS = [u for i, u in enumerate(UNITS) if i % 2 == 0][:NCU]
_cs = set(CACHED_UNITS)
UNCACHED_UNITS = [u for u in UNITS if u not in _cs]
# ohcache column layout: cached units' groups first, then uncached
OHGB = {}
_g2 = 0
for _u in CACHED_UNITS + UNCACHED_UNITS:
    OHGB[_u] = _g2
    _g2 += UNIT_NIDX[_u] // 128
CGROUPS = sum(UNIT_NIDX[u] // 128 for u in CACHED_UNITS)

_PROGRAM = None
# tuned pipeline depths (see session notes: gather/one-hot prefetch depth
# dominates; PSUM is bank-granular so pp+hn+hnT+pooled must total <= 8)
SKIP_OH = SKIP_FIN = SKIP_PART = SKIP_G = False
GB = 9      # gather destination buffers (deep SWDGE pipeline)
OHB = 12    # one-hot load buffers (deep HWDGE prefetch)
PPB = 4     # PSUM accumulation banks
OH_ENG = "sync"
OHPRI = GPRI = 0
GQN = 4     # SWDGE queues used round-robin by gathers
AG_DECOUPLE = False
SP1 = False
AGMODE = 2
REPEAT = 1


def _preprocess(inp: dict):
    batch = np.asarray(inp["batch"])
    ei = np.asarray(inp["edge_index"])
    ew = np.asarray(inp["edge_attr"], dtype=np.float32)
    x = np.asarray(inp["x"], dtype=np.float32)
    src, dst = ei[0].astype(np.int64), ei[1].astype(np.int64)

    bounds = np.searchsorted(batch, np.arange(0, B + 1, GPC)).astype(np.int64)
    sizes = np.diff(bounds)
    assert sizes.max() <= S, f"shard overflow: {sizes.max()} > {S}"

    node = np.arange(N_NODES, dtype=np.int64)
    core_of = (np.searchsorted(bounds, node, side="right") - 1).astype(np.int64)
    off = node - bounds[core_of]
    # chunk-major table: row = chunk*CHROWS + core*CHS + (off % CHS)
    tab = (off // CHS) * CHROWS + core_of * CHS + (off % CHS)

    deg = (np.bincount(dst, weights=ew.astype(np.float64), minlength=N_NODES) + 1.0)
    deg = deg.astype(np.float32)
    dinv = 1.0 / np.sqrt(deg)
    norm = (dinv[src] * ew * dinv[dst]).astype(np.float32)
    dinv2 = (1.0 / deg).astype(np.float32)

    # append self-edges: src=dst=n with weight 1/deg(n)
    src_a = np.concatenate([src, node])
    dst_a = np.concatenate([dst, node])
    norm_a = np.concatenate([norm, dinv2])

    # full replicated x table (node-major, bf16)
    xtab = np.zeros((TAB, F), dtype=ml_dtypes.bfloat16)
    xtab[tab] = x.astype(ml_dtypes.bfloat16)

    identf = np.eye(128, dtype=np.float32)
    identb = np.eye(128, dtype=ml_dtypes.bfloat16)

    # weights
    conv_w = np.asarray(inp["conv_w"], dtype=np.float32).reshape(6, F, F)
    convw = (conv_w.transpose(1, 0, 2).reshape(F, 6 * F)
             / OHSCALE).astype(ml_dtypes.bfloat16)
    convb = np.asarray(inp["conv_b"], dtype=np.float32).reshape(6, F).T.copy()
    jk_w = np.asarray(inp["jk_w"], dtype=np.float32).reshape(NB, 2, F, F).reshape(6, F, F)
    jkw = jk_w.transpose(1, 0, 2).reshape(F, 6 * F).astype(ml_dtypes.bfloat16)
    jkb = np.asarray(inp["jk_b"], dtype=np.float32).T.copy()
    sc = (np.asarray(inp["bn_gamma"], dtype=np.float32)
          / np.sqrt(np.asarray(inp["bn_var"], dtype=np.float32) + BN_EPS))
    tr = (np.asarray(inp["bn_beta"], dtype=np.float32)
          - np.asarray(inp["bn_mean"], dtype=np.float32) * sc)
    bns = sc.reshape(NB, F).T.copy()
    bnt = tr.reshape(NB, F).T.copy()
    lin1_w = np.asarray(inp["lin1_w"], dtype=np.float32).reshape(NB, F, F)
    l1w = lin1_w.transpose(1, 0, 2).reshape(F, NB * F).copy()
    l1b = np.asarray(inp["lin1_b"], dtype=np.float32).reshape(F, 1).copy()
    l2w = np.asarray(inp["lin2_w"], dtype=np.float32).copy()
    l2b = np.asarray(inp["lin2_b"], dtype=np.float32).reshape(10, 1).copy()

    shared = {
        "xtab": xtab, "identf": identf, "identb": identb,
        "convw": convw, "convb": convb, "jkw": jkw, "jkb": jkb,
        "bns": bns, "bnt": bnt, "l1w": l1w, "l1b": l1b, "l2w": l2w, "l2b": l2b,
    }

    dst_core = core_of[dst_a]
    dst_off = off[dst_a]
    src_tab = tab[src_a]

    # per-bucket slot bases in the unit-major layout
    bucket_base = np.empty(NW * NCH, dtype=np.int64)  # bid = w*NCH + k
    bucket_cap = np.empty(NW * NCH, dtype=np.int64)
    for k, wp in UNITS:
        cha, chb = UNIT_CH[(k, wp)]
        ub = UNIT_SLOT[(k, wp)]
        bucket_base[(2 * wp) * NCH + k] = ub
        bucket_base[(2 * wp + 1) * NCH + k] = ub + cha * 128
        bucket_cap[(2 * wp) * NCH + k] = cha * 128
        bucket_cap[(2 * wp + 1) * NCH + k] = chb * 128

    in_maps = []
    for c in range(NC):
        eidx = np.flatnonzero(dst_core == c)
        e_w = dst_off[eidx] // 128
        e_k = src_tab[eidx] // CHROWS
        bid = e_w * NCH + e_k
        order = np.argsort(bid, kind="stable")
        eidx = eidx[order]
        bid = bid[order]
        counts = np.bincount(bid, minlength=NW * NCH)
        assert (counts <= bucket_cap).all(), (
            f"bucket overflow core {c}: {counts.max()}")
        starts = np.concatenate([[0], np.cumsum(counts)])[:-1]
        pos = np.arange(len(eidx)) - starts[bid]
        slot = bucket_base[bid] + pos

        idx_slots = np.zeros(NSLOTS, dtype=np.int64)
        idx_slots[slot] = src_tab[eidx] % CHROWS
        rel_slots = np.zeros(NSLOTS, dtype=np.int64)
        rel_slots[slot] = dst_off[eidx] % 128
        nrm_slots = np.zeros(NSLOTS, dtype=np.float32)
        nrm_slots[slot] = norm_a[eidx]

        # wrapped gather idx: per unit [16, nidx/16], concatenated, tiled x8
        gcols = []
        for u in UNITS:
            n = UNIT_NIDX[u]
            runs = idx_slots[UNIT_SLOT[u]:UNIT_SLOT[u] + n]
            gcols.append(runs.reshape(n // 16, 16).T)
        gidx = np.tile(np.concatenate(gcols, axis=1).astype(np.int16), (8, 1))

        # host-built scaled one-hot tiles: [128, NGROUPS*128] fp8, with the
        # cached units' groups packed first (SBUF-resident across convs)
        oh = np.zeros((NGROUPS, 128, 128), dtype=np.float32)
        grp = slot // 128
        oh[grp, slot % 128, rel_slots[slot]] = nrm_slots[slot] * OHSCALE
        perm = np.empty(NGROUPS, dtype=np.int64)
        for u in UNITS:
            n = UNIT_NIDX[u] // 128
            perm[OHGB[u]:OHGB[u] + n] = np.arange(UNIT_GBASE[u],
                                                  UNIT_GBASE[u] + n)
        ohc = np.ascontiguousarray(oh[perm].transpose(1, 0, 2)).reshape(
            128, NGROUPS * 128).astype(ml_dtypes.float8_e4m3)

        # per-graph pooling one-hot
        ln = np.arange(sizes[c], dtype=np.int64)
        pool = np.zeros((128, NW * GPC), dtype=ml_dtypes.bfloat16)
        g_of = batch[bounds[c] + ln].astype(np.int64) - c * GPC
        pool[ln % 128, (ln // 128) * GPC + g_of] = 1.0

        m = {"gidx": gidx, "ohcache": ohc, "pool": pool}
        m.update(shared)
        in_maps.append(m)
    return in_maps


def _build_program(stage=99):
    nc = bacc.Bacc("TRN2", target_bir_lowering=False, debug=False,
                   num_devices=NC, num_swdge_queues=NQ)
    AF = mybir.ActivationFunctionType
    OP = mybir.AluOpType

    ap = {}
    big_inputs = ([] if stage == 0 else [
        ("xtab", [TAB, F], bf16),
        ("gidx", [128, GIDXCOLS], i16),
        ("ohcache", [128, NGROUPS * 128], fp8),
        ("pool", [128, NW * GPC], bf16),
    ])
    for name, shape, dt in big_inputs + [
        ("identf", [128, 128], f32), ("identb", [128, 128], bf16),
        ("convw", [F, 6 * F], bf16), ("convb", [F, 6], f32),
        ("jkw", [F, 6 * F], bf16), ("jkb", [F, NB], f32),
        ("bns", [F, NB], f32), ("bnt", [F, NB], f32),
        ("l1w", [F, NB * F], f32), ("l1b", [F, 1], f32),
        ("l2w", [F, 10], f32), ("l2b", [10, 1], f32),
    ]:
        ap[name] = nc.dram_tensor(name, shape, dt, kind="ExternalInput").ap()
    out_ap = nc.dram_tensor("out", [GPC, 10], f32, kind="ExternalOutput").ap()

    with tile.TileContext(nc) as tc:
        with (
            tc.tile_pool(name="dram", bufs=1, space="DRAM") as dram,
            tc.tile_pool(name="pers", bufs=1) as pers,
            tc.tile_pool(name="rot", bufs=1) as rot,
            tc.tile_pool(name="psum", bufs=1, space="PSUM") as psum,
        ):
            ag_in = dram.tile([S, F], bf16)
            ag_const = dram.tile([S, F], bf16)

            # ---- persistent SBUF loads
            sb = {}
            for name in ((["gidx", "pool"] if stage > 0 else []) +
                         ["identf", "identb", "convw", "convb",
                          "jkw", "jkb", "bns", "bnt", "l1w", "l1b", "l2w",
                          "l2b"]):
                t_ = pers.tile(list(ap[name].shape), ap[name].dtype,
                               name=f"sb_{name}")
                nc.sync.dma_start(t_[:], ap[name][:])
                sb[name] = t_

            h1_fm = pers.tile([128, S], bf16, name="h1_fm")
            if SKIP_FIN:
                nc.scalar.copy(h1_fm[:, 0:128], sb["identb"][:])
            h2_fm = pers.tile([128, S], bf16, name="h2_fm")
            hb_fm = pers.tile([128, S], bf16, name="hb_fm")
            part = pers.tile([128, NW, 128], bf16, name="part")
            z_sb = pers.tile([128, NB, GPC], f32, name="z_sb")

            qctr = [0]
            _cset = set(CACHED_UNITS)
            if stage == 0:
                ohsb = None
            if AG_DECOUPLE:
                for _w in range(NW):
                    nc.sync.dma_start(ag_const[_w * 128:(_w + 1) * 128, :],
                                      sb["identb"][:])
            if stage > 0:
                Gpre = pers.tile([128, MAXG, F], bf16, name="Gpre")
                nc.sync.dma_start(Gpre[:, 0:1, :], ap["xtab"][0:128, :])
                OHpre = pers.tile([128, MAXG, 128], fp8, name="OHpre")
                nc.sync.dma_start(OHpre[:, 0:1, :], ap["ohcache"][:, 0:128])
            if CGROUPS and stage > 0:
                ohsb = pers.tile([128, CGROUPS, 128], fp8, name="ohsb")
                nc.sync.dma_start(ohsb[:], ap["ohcache"][:, 0:CGROUPS * 128])
            else:
                ohsb = None

            def unit_tiles(tables, k, wp):
                u = (k, wp)
                ng = UNIT_NIDX[u] // 128
                if SKIP_G:
                    G = Gpre
                else:
                    from contextlib import nullcontext
                    G = rot.tile([128, MAXG, F], bf16, tag="G", bufs=GB, name="G")
                    with (tc.high_priority(offset=GPRI) if GPRI
                          else nullcontext()):
                        nc.gpsimd.dma_gather(
                            out_ap=G[:, 0:ng, :], in_ap=tables[k][:],
                            idxs_ap=sb["gidx"][:, UNIT_GCOL[u]:
                                               UNIT_GCOL[u] + UNIT_NIDX[u] // 16],
                            num_idxs=UNIT_NIDX[u], num_idxs_reg=UNIT_NIDX[u],
                            elem_size=F, single_packet=SP1,
                            queue_num=qctr[0] % GQN)
                    qctr[0] += 1
                if SKIP_OH:
                    OH, ob = OHpre, 0
                elif u in _cset:
                    OH, ob = ohsb, OHGB[u]
                else:
                    from contextlib import nullcontext
                    OH = rot.tile([128, MAXG, 128], fp8, tag="OH", bufs=OHB,
                                  name="OH")
                    src_ap = ap["ohcache"][:, OHGB[u] * 128:
                                           (OHGB[u] + ng) * 128]
                    with (tc.high_priority(offset=OHPRI) if OHPRI
                          else nullcontext()):
                        if OH_ENG == "gps3":
                            nc.gpsimd.dma_start(OH[:, 0:ng, :], src_ap,
                                                queue_num=NQ - 1)
                        elif OH_ENG == "act":
                            nc.scalar.dma_start(OH[:, 0:ng, :], src_ap)
                        else:
                            nc.sync.dma_start(OH[:, 0:ng, :], src_ap)
                    ob = 0
                return G, OH, ob

            def ag_chunk(i, k):
                src = ag_const if AG_DECOUPLE else ag_in
                tk = dram.tile([CHROWS, F], bf16, addr_space="Shared",
                               tag=f"t{_rep[0]}_{i}_{k}",
                               name=f"t{_rep[0]}_{i}_{k}")
                nc.gpsimd.collective_compute(
                    "AllGather", OP.bypass,
                    replica_groups=[list(range(NC))],
                    ins=[src[k * CHS:(k + 1) * CHS, :].opt()],
                    outs=[tk.opt()])
                # gathers from Shared-space DRAM run ~60% slower than from
                # regular DRAM; copy the table out (split across both HWDGE
                # rings to halve the serial latency) before gathering from it
                tl = dram.tile([CHROWS, F], bf16,
                               tag=f"l{_rep[0]}_{i}_{k}",
                               name=f"l{_rep[0]}_{i}_{k}")
                h = CHROWS // 2
                nc.sync.dma_start(tl[0:h, :], tk[0:h, :])
                nc.scalar.dma_start(tl[h:, :], tk[h:, :])
                return tl

            def ag_full(i):
                # single collective for the whole shard (core-major output),
                # reshuffled to the 2 chunk-major tables during the copy-out
                src = ag_const if AG_DECOUPLE else ag_in
                tk = dram.tile([NC * S, F], bf16, addr_space="Shared",
                               tag=f"t{_rep[0]}_{i}", name=f"t{_rep[0]}_{i}")
                nc.gpsimd.collective_compute(
                    "AllGather", OP.bypass,
                    replica_groups=[list(range(NC))],
                    ins=[src[:].opt()], outs=[tk.opt()])
                tabs = []
                for k in range(NCH):
                    tl = dram.tile([CHROWS, F], bf16,
                                   tag=f"l{_rep[0]}_{i}_{k}",
                                   name=f"l{_rep[0]}_{i}_{k}")
                    for co in range(NC):
                        eng = nc.sync if co % 2 == 0 else nc.scalar
                        eng.dma_start(
                            tl[co * CHS:(co + 1) * CHS, :],
                            tk[co * S + k * CHS:co * S + (k + 1) * CHS, :])
                    tabs.append(tl)
                return tabs

            def conv(lk, tables, h_out, ag, jkli=None):
                if jkli is not None:
                    pooled = psum.tile([128, GPC], f32, tag="pooled", bufs=1,
                                       name="pooled")
                # chunk-0 pass: accumulate in PSUM, evict partials to SBUF
                for wp in range(NWP):
                    G, OH, ob = unit_tiles(tables, 0, wp)
                    cha, chb = UNIT_CH[(0, wp)]
                    for half, ch in ((0, cha), (1, chb)):
                        w = 2 * wp + half
                        gb = half * cha
                        pp = psum.tile([128, 128], f32, tag="pp", bufs=PPB,
                                       name="pp")
                        for c in range(ch):
                            nc.tensor.matmul(pp[:], G[:, gb + c, :],
                                             OH[:, ob + gb + c, :],
                                             start=(c == 0), stop=(c == ch - 1))
                        if not SKIP_PART:
                            nc.scalar.copy(part[:, w, :], pp[:])
                # chunk-1 pass: re-inject partial, finish, apply W+bias+ReLU
                tabs = []
                for wp in range(NWP):
                    G, OH, ob = unit_tiles(tables, 1, wp)
                    cha, chb = UNIT_CH[(1, wp)]
                    for half, ch in ((0, cha), (1, chb)):
                        w = 2 * wp + half
                        gb = half * cha
                        pp = psum.tile([128, 128], f32, tag="pp", bufs=PPB,
                                       name="pp")
                        if SKIP_PART:
                            nc.tensor.matmul(pp[:], sb["identb"][:],
                                             sb["identb"][:],
                                             start=True, stop=False)
                        else:
                            nc.tensor.matmul(pp[:], sb["identb"][:],
                                             part[:, w, :],
                                             start=True, stop=False)
                        for c in range(ch):
                            nc.tensor.matmul(pp[:], G[:, gb + c, :],
                                             OH[:, ob + gb + c, :],
                                             start=False, stop=(c == ch - 1))
                        if SKIP_FIN:
                            continue
                        aggs = rot.tile([128, 128], bf16, tag="aggs", bufs=4,
                                        name="aggs")
                        nc.scalar.copy(aggs[:], pp[:])
                        hn = psum.tile([128, 128], f32, tag="hn", bufs=2,
                                       name="hn")
                        nc.tensor.matmul(hn[:], sb["convw"][:, lk * F:(lk + 1) * F],
                                         aggs[:], start=True, stop=True)
                        nc.scalar.activation(h_out[:, w * 128:(w + 1) * 128],
                                             hn[:], AF.Relu,
                                             bias=sb["convb"][:, lk:lk + 1])
                        if jkli is not None:
                            # fused JumpingKnowledge: hb = relu(jk_w @ [h1;h2])
                            li = jkli
                            hb = psum.tile([128, 128], f32, tag="hn", bufs=2,
                                           name="hb")
                            nc.tensor.matmul(
                                hb[:],
                                sb["jkw"][:, (2 * li) * F:(2 * li + 1) * F],
                                h1_fm[:, w * 128:(w + 1) * 128],
                                start=True, stop=False)
                            nc.tensor.matmul(
                                hb[:],
                                sb["jkw"][:, (2 * li + 1) * F:(2 * li + 2) * F],
                                h_out[:, w * 128:(w + 1) * 128],
                                start=False, stop=True)
                            nc.scalar.activation(hb_fm[:, w * 128:(w + 1) * 128],
                                                 hb[:], AF.Relu,
                                                 bias=sb["jkb"][:, li:li + 1])
                            src_fm = hb_fm
                        else:
                            src_fm = h_out
                        if ag is not None or jkli is not None:
                            hnT = psum.tile([128, 128], bf16, tag="hnT",
                                            bufs=1, name="hnT")
                            nc.tensor.transpose(hnT[:],
                                                src_fm[:, w * 128:(w + 1) * 128],
                                                sb["identb"][:])
                            hcol = rot.tile([128, 128], bf16, tag="hcol",
                                            bufs=4, name="hcol")
                            nc.scalar.copy(hcol[:], hnT[:])
                        if jkli is not None:
                            nc.tensor.matmul(pooled[:], hcol[:],
                                             sb["pool"][:, w * GPC:(w + 1) * GPC],
                                             start=(w == 0), stop=(w == NW - 1))
                        if ag is not None:
                            nc.sync.dma_start(ag_in[w * 128:(w + 1) * 128, :],
                                              hcol[:])
                            # trigger the chunk AG as soon as its input rows
                            # are all written (w24 -> chunk0, w49 -> chunk1)
                            if AGMODE == 2:
                                if w == NW // 2 - 1:
                                    tabs.append(ag_chunk(ag, 0))
                                elif w == NW - 1:
                                    tabs.append(ag_chunk(ag, 1))
                            elif w == NW - 1:
                                tabs.extend(ag_full(ag))
                if jkli is not None:
                    nc.scalar.copy(z_sb[:, jkli, :], pooled[:])
                return tabs if ag is not None else None

            def jk(li, ag):
                last = ag is None
                tabs = []
                pooled = psum.tile([128, GPC], f32, tag="pooled", bufs=1,
                                   name="pooled")
                for w in range(NW):
                    hb = psum.tile([128, 128], f32, tag="hn", bufs=2,
                                   name="hb")
                    nc.tensor.matmul(hb[:],
                                     sb["jkw"][:, (2 * li) * F:(2 * li + 1) * F],
                                     h1_fm[:, w * 128:(w + 1) * 128],
                                     start=True, stop=False)
                    nc.tensor.matmul(hb[:],
                                     sb["jkw"][:, (2 * li + 1) * F:(2 * li + 2) * F],
                                     h2_fm[:, w * 128:(w + 1) * 128],
                                     start=False, stop=True)
                    nc.scalar.activation(hb_fm[:, w * 128:(w + 1) * 128],
                                         hb[:], AF.Relu,
                                         bias=sb["jkb"][:, li:li + 1])
                    hnT = psum.tile([128, 128], bf16, tag="hnT", bufs=1,
                                    name="hnT")
                    nc.tensor.transpose(hnT[:], hb_fm[:, w * 128:(w + 1) * 128],
                                        sb["identb"][:])
                    hcol = rot.tile([128, 128], bf16, tag="hcol", bufs=4,
                                    name="hcol")
                    nc.scalar.copy(hcol[:], hnT[:])
                    if not last:
                        nc.sync.dma_start(ag_in[w * 128:(w + 1) * 128, :],
                                          hcol[:])
                        if AGMODE == 2:
                            if w == NW // 2 - 1:
                                tabs.append(ag_chunk(ag, 0))
                            elif w == NW - 1:
                                tabs.append(ag_chunk(ag, 1))
                        elif w == NW - 1:
                            tabs.extend(ag_full(ag))
                    nc.tensor.matmul(pooled[:], hcol[:],
                                     sb["pool"][:, w * GPC:(w + 1) * GPC],
                                     start=(w == 0), stop=(w == NW - 1))
                nc.scalar.copy(z_sb[:, li, :], pooled[:])
                return tabs if not last else None

            # ---- main flow
            steps = [
                lambda: conv(0, [ap["xtab"][k * CHROWS:(k + 1) * CHROWS, :]
                                 for k in range(NCH)], h1_fm, 0),
                lambda t: conv(1, t, h2_fm, 1, jkli=0),
                lambda t: conv(2, t, h1_fm, 2),
                lambda t: conv(3, t, h2_fm, 3, jkli=1),
                lambda t: conv(4, t, h1_fm, 4),
                lambda t: conv(5, t, h2_fm, None, jkli=2),
            ]
            _rep = [0]
            for rep in range(REPEAT):
                _rep[0] = rep
                table = None
                for i, st in enumerate(steps):
                    if i >= stage:
                        break
                    r = st(table) if st.__code__.co_argcount else st()
                    if r is not None:
                        table = r

            # ---- head
            if stage < 6:
                outt0 = rot.tile([GPC, 10], f32, tag="outt", bufs=1,
                                 name="outt0")
                nc.vector.tensor_copy(outt0[:], h1_fm[0:GPC, 0:10])
                nc.sync.dma_start(out_ap[:], outt0[:])
            else:
                _head(nc, tc, rot, psum, sb, z_sb, out_ap)

    nc.compile()
    return nc


def _head(nc, tc, rot, psum, sb, z_sb, out_ap):
    AF = mybir.ActivationFunctionType
    OP = mybir.AluOpType
    zbn = rot.tile([128, NB, GPC], f32, tag="zbn", bufs=1, name="zbn")
    for t in range(NB):
        nc.vector.tensor_scalar(
            out=zbn[:, t, :], in0=z_sb[:, t, :],
            scalar1=sb["bns"][:, t:t + 1], scalar2=sb["bnt"][:, t:t + 1],
            op0=OP.mult, op1=OP.add)
    a1 = psum.tile([128, GPC], f32, tag="hn", bufs=2, name="a1")
    for t in range(NB):
        nc.tensor.matmul(a1[:], sb["l1w"][:, t * F:(t + 1) * F],
                         zbn[:, t, :], start=(t == 0), stop=(t == NB - 1))
    a1s = rot.tile([128, GPC], f32, tag="a1s", bufs=1, name="a1s")
    nc.scalar.activation(a1s[:], a1[:], AF.Relu, bias=sb["l1b"][:])
    z2 = psum.tile([10, GPC], f32, tag="pooled", bufs=1, name="z2")
    nc.tensor.matmul(z2[:], sb["l2w"][:], a1s[:], start=True, stop=True)
    z2s = rot.tile([10, GPC], f32, tag="z2s", bufs=1, name="z2s")
    nc.scalar.activation(z2s[:], z2[:], AF.Identity, bias=sb["l2b"][:])
    z2T = psum.tile([GPC, 10], f32, tag="hnT", bufs=1, name="z2T")
    nc.tensor.transpose(z2T[:], z2s[:], sb["identf"][0:10, 0:10])
    z2Ts = rot.tile([GPC, 10], f32, tag="z2Ts", bufs=1, name="z2Ts")
    nc.vector.tensor_copy(z2Ts[:], z2T[:])
    negm = rot.tile([GPC, 1], f32, tag="negm", bufs=1, name="negm")
    nc.vector.tensor_reduce(negm[:], z2Ts[:], mybir.AxisListType.X,
                            OP.max, negate=True)
    et = rot.tile([GPC, 10], f32, tag="et", bufs=1, name="et")
    nc.scalar.activation(et[:], z2Ts[:], AF.Exp, bias=negm[:])
    ssum = rot.tile([GPC, 1], f32, tag="ssum", bufs=1, name="ssum")
    nc.vector.tensor_reduce(ssum[:], et[:], mybir.AxisListType.X, OP.add)
    rcp = rot.tile([GPC, 1], f32, tag="rcp", bufs=1, name="rcp")
    nc.vector.reciprocal(rcp[:], ssum[:])
    outt = rot.tile([GPC, 10], f32, tag="outt", bufs=1, name="outt")
    nc.vector.tensor_scalar_mul(outt[:], et[:], rcp[:])
    nc.sync.dma_start(out_ap[:], outt[:])


def _get_program():
    global _PROGRAM
    if _PROGRAM is None:
        _PROGRAM = _build_program()
    return _PROGRAM


def kernel(**inputs) -> np.ndarray:
    in_maps = _preprocess(inputs)
    nc = _get_program()
    res = run_bass_kernel_spmd(nc, in_maps, list(range(NC)))
    return np.concatenate([res.results[c]["out"] for c in range(NC)], axis=0)


# revision 25
# speedup vs baseline: 1.2340x; 1.2156x over previous
"""Trainium2 Bass kernel for nn_BaseModel_14499809591724 (GNN message passing).

Strategy (8 NeuronCores, data-parallel over graph batches):
  - Nodes split into 8 contiguous shards at graph boundaries (batch sorted),
    padded to S=6400 rows; replicated node table = [8*S, 128] bf16, stored
    chunk-major in 2 chunks of 25600 rows (int16 gather-index safe).
  - Self-loops are folded into the edge list as explicit (n -> n) edges with
    weight 1/deg, so a conv is purely gather + one-hot scatter matmul.
  - Per GCN conv, per window-pair (2 x 128 dst nodes) and table chunk: one
    dma_gather pulls the source rows (round-robin over 4 SWDGE queues so Q7
    descriptor generation runs 4-wide); the matching scaled one-hot tiles
    (host-precomputed, bf16) stream in from DRAM; PE contracts
    aggT[f,dst] += G[e,f]^T @ OH[e,dst] in PSUM.  Chunk-0 partials are
    evicted to SBUF by ACT and re-injected via an identity matmul in the
    chunk-1 pass.  W/bias/ReLU apply feature-major with no transposes; only
    convs feeding an AllGather transpose back to node-major.
  - The vector engine is kept COLD during convs: its SBUF port contends with
    GPSIMD's descriptor generation and halves gather throughput.
  - After each conv that feeds another conv, the 8 shards are AllGathered
    (bf16, 2 chunks, overlapped with compute).
  - JumpingKnowledge + per-graph pooling (one-hot matmul) + BN + MLP head +
    softmax run per core on its own 64 graphs; host concatenates 8 x [64,10].
"""
import sys
import numpy as np
import ml_dtypes

sys.path.insert(0, "/opt/trn_rl_repo")

from concourse import bacc, tile, mybir  # noqa: E402
from concourse.bass_utils import run_bass_kernel_spmd  # noqa: E402

# ---- model / sharding constants (shapes fixed by the problem) ----
NC = 8
N_NODES = 50000
N_EDGES = 800000
F = 128
B = 512
GPC = B // NC          # graphs per core = 64
S = 6400               # padded nodes per shard (max real shard is 6368)
NW = S // 128          # 50 windows per core
NWP = NW // 2          # 25 window pairs
TAB = NC * S           # 51200 table rows
NCH = 2                # table chunks (progressive AllGather pipeline)
CHS = S // NCH         # 3200 shard rows per chunk
CHROWS = NC * CHS      # 25600 table rows per chunk (int16-safe)
NB = 3
BN_EPS = 1e-5
NQ = 4                 # SWDGE queues for gather descriptor generation
OHSCALE = 128.0        # one-hot fp8 pre-scale; 1/OHSCALE folded into conv_w

f32 = mybir.dt.float32
bf16 = mybir.dt.bfloat16
i16 = mybir.dt.int16
fp8 = mybir.dt.float8e4


def _ch_of(w: int, k: int) -> int:
    """Groups for bucket (window w, chunk k). The chunk holding w's own nodes
    also receives w's 128 self-edges, so it gets one extra group."""
    return 10 if k == (0 if w < NW // 2 else 1) else 9


# static slot/group layout: units are (k, wp) in k-major order; each unit is
# one gather covering buckets (2wp, k) then (2wp+1, k).
UNITS = [(k, wp) for k in range(NCH) for wp in range(NWP)]
UNIT_CH = {(k, wp): (_ch_of(2 * wp, k), _ch_of(2 * wp + 1, k)) for k, wp in UNITS}
UNIT_NIDX = {u: (UNIT_CH[u][0] + UNIT_CH[u][1]) * 128 for u in UNITS}
UNIT_SLOT = {}
UNIT_GBASE = {}
UNIT_GCOL = {}
_s = _g = _c = 0
for _u in UNITS:
    UNIT_SLOT[_u] = _s
    UNIT_GBASE[_u] = _g
    UNIT_GCOL[_u] = _c
    _s += UNIT_NIDX[_u]
    _g += UNIT_NIDX[_u] // 128
    _c += UNIT_NIDX[_u] // 16
NSLOTS = _s            # 121600
NGROUPS = _g           # 950
GIDXCOLS = _c          # 7600
MAXG = max(UNIT_NIDX[u] // 128 for u in UNITS)  # 20

NCU = 16               # OH units cached persistently in SBUF
CACHED_UNITS = [u for i, u in enumerate(UNITS) if i % 2 == 0][:NCU]
_cs = set(CACHED_UNITS)
UNCACHED_UNITS = [u for u in UNITS if u not in _cs]
# ohcache column layout: cached units' groups first, then uncached
OHGB = {}
_g2 = 0
for _u in CACHED_UNITS + UNCACHED_UNITS:
    OHGB[_u] = _g2
    _g2 += UNIT_NIDX[_u] // 128
CGROUPS = sum(UNIT_NIDX[u] // 128 for u in CACHED_UNITS)

_PROGRAM = None
# tuned pipeline depths (see session notes: gather/one-hot prefetch depth
# dominates; PSUM is bank-granular so pp+hn+hnT+pooled must total <= 8)
SKIP_OH = SKIP_FIN = SKIP_PART = SKIP_G = False
GB = 9      # gather destination buffers (deep SWDGE pipeline)
OHB = 12    # one-hot load buffers (deep HWDGE prefetch)
PPB = 4     # PSUM accumulation banks
OH_ENG = "sync"
OHPRI = GPRI = 0
GQN = 4     # SWDGE queues used round-robin by gathers
AG_DECOUPLE = False
SP1 = False
AGMODE = 2
REPEAT = 1


def _preprocess(inp: dict):
    batch = np.asarray(inp["batch"])
    ei = np.asarray(inp["edge_index"])
    ew = np.asarray(inp["edge_attr"], dtype=np.float32)
    x = np.asarray(inp["x"], dtype=np.float32)
    src, dst = ei[0].astype(np.int64), ei[1].astype(np.int64)

    bounds = np.searchsorted(batch, np.arange(0, B + 1, GPC)).astype(np.int64)
    sizes = np.diff(bounds)
    assert sizes.max() <= S, f"shard overflow: {sizes.max()} > {S}"

    node = np.arange(N_NODES, dtype=np.int64)
    core_of = (np.searchsorted(bounds, node, side="right") - 1).astype(np.int64)
    off = node - bounds[core_of]
    # chunk-major table: row = chunk*CHROWS + core*CHS + (off % CHS)
    tab = (off // CHS) * CHROWS + core_of * CHS + (off % CHS)

    deg = (np.bincount(dst, weights=ew.astype(np.float64), minlength=N_NODES) + 1.0)
    deg = deg.astype(np.float32)
    dinv = 1.0 / np.sqrt(deg)
    norm = (dinv[src] * ew * dinv[dst]).astype(np.float32)
    dinv2 = (1.0 / deg).astype(np.float32)

    # append self-edges: src=dst=n with weight 1/deg(n)
    src_a = np.concatenate([src, node])
    dst_a = np.concatenate([dst, node])
    norm_a = np.concatenate([norm, dinv2])

    # full replicated x table (node-major, bf16)
    xtab = np.zeros((TAB, F), dtype=ml_dtypes.bfloat16)
    xtab[tab] = x.astype(ml_dtypes.bfloat16)

    identf = np.eye(128, dtype=np.float32)
    identb = np.eye(128, dtype=ml_dtypes.bfloat16)

    # weights
    conv_w = np.asarray(inp["conv_w"], dtype=np.float32).reshape(6, F, F)
    convw = (conv_w.transpose(1, 0, 2).reshape(F, 6 * F)
             / OHSCALE).astype(ml_dtypes.bfloat16)
    convb = np.asarray(inp["conv_b"], dtype=np.float32).reshape(6, F).T.copy()
    jk_w = np.asarray(inp["jk_w"], dtype=np.float32).reshape(NB, 2, F, F).reshape(6, F, F)
    jkw = jk_w.transpose(1, 0, 2).reshape(F, 6 * F).astype(ml_dtypes.bfloat16)
    jkb = np.asarray(inp["jk_b"], dtype=np.float32).T.copy()
    sc = (np.asarray(inp["bn_gamma"], dtype=np.float32)
          / np.sqrt(np.asarray(inp["bn_var"], dtype=np.float32) + BN_EPS))
    tr = (np.asarray(inp["bn_beta"], dtype=np.float32)
          - np.asarray(inp["bn_mean"], dtype=np.float32) * sc)
    bns = sc.reshape(NB, F).T.copy()
    bnt = tr.reshape(NB, F).T.copy()
    lin1_w = np.asarray(inp["lin1_w"], dtype=np.float32).reshape(NB, F, F)
    l1w = lin1_w.transpose(1, 0, 2).reshape(F, NB * F).copy()
    l1b = np.asarray(inp["lin1_b"], dtype=np.float32).reshape(F, 1).copy()
    l2w = np.asarray(inp["lin2_w"], dtype=np.float32).copy()
    l2b = np.asarray(inp["lin2_b"], dtype=np.float32).reshape(10, 1).copy()

    shared = {
        "xtab": xtab, "identf": identf, "identb": identb,
        "convw": convw, "convb": convb, "jkw": jkw, "jkb": jkb,
        "bns": bns, "bnt": bnt, "l1w": l1w, "l1b": l1b, "l2w": l2w, "l2b": l2b,
    }

    dst_core = core_of[dst_a]
    dst_off = off[dst_a]
    src_tab = tab[src_a]

    # per-bucket slot bases in the unit-major layout
    bucket_base = np.empty(NW * NCH, dtype=np.int64)  # bid = w*NCH + k
    bucket_cap = np.empty(NW * NCH, dtype=np.int64)
    for k, wp in UNITS:
        cha, chb = UNIT_CH[(k, wp)]
        ub = UNIT_SLOT[(k, wp)]
        bucket_base[(2 * wp) * NCH + k] = ub
        bucket_base[(2 * wp + 1) * NCH + k] = ub + cha * 128
        bucket_cap[(2 * wp) * NCH + k] = cha * 128
        bucket_cap[(2 * wp + 1) * NCH + k] = chb * 128

    in_maps = []
    for c in range(NC):
        eidx = np.flatnonzero(dst_core == c)
        e_w = dst_off[eidx] // 128
        e_k = src_tab[eidx] // CHROWS
        bid = e_w * NCH + e_k
        order = np.argsort(bid, kind="stable")
        eidx = eidx[order]
        bid = bid[order]
        counts = np.bincount(bid, minlength=NW * NCH)
        assert (counts <= bucket_cap).all(), (
            f"bucket overflow core {c}: {counts.max()}")
        starts = np.concatenate([[0], np.cumsum(counts)])[:-1]
        pos = np.arange(len(eidx)) - starts[bid]
        slot = bucket_base[bid] + pos

        idx_slots = np.zeros(NSLOTS, dtype=np.int64)
        idx_slots[slot] = src_tab[eidx] % CHROWS
        rel_slots = np.zeros(NSLOTS, dtype=np.int64)
        rel_slots[slot] = dst_off[eidx] % 128
        nrm_slots = np.zeros(NSLOTS, dtype=np.float32)
        nrm_slots[slot] = norm_a[eidx]

        # wrapped gather idx: per unit [16, nidx/16], concatenated, tiled x8
        gcols = []
        for u in UNITS:
            n = UNIT_NIDX[u]
            runs = idx_slots[UNIT_SLOT[u]:UNIT_SLOT[u] + n]
            gcols.append(runs.reshape(n // 16, 16).T)
        gidx = np.tile(np.concatenate(gcols, axis=1).astype(np.int16), (8, 1))

        # host-built scaled one-hot tiles: [128, NGROUPS*128] fp8, with the
        # cached units' groups packed first (SBUF-resident across convs)
        oh = np.zeros((NGROUPS, 128, 128), dtype=np.float32)
        grp = slot // 128
        oh[grp, slot % 128, rel_slots[slot]] = nrm_slots[slot] * OHSCALE
        perm = np.empty(NGROUPS, dtype=np.int64)
        for u in UNITS:
            n = UNIT_NIDX[u] // 128
            perm[OHGB[u]:OHGB[u] + n] = np.arange(UNIT_GBASE[u],
                                                  UNIT_GBASE[u] + n)
        ohc = np.ascontiguousarray(oh[perm].transpose(1, 0, 2)).reshape(
            128, NGROUPS * 128).astype(ml_dtypes.float8_e4m3)

        # per-graph pooling one-hot
        ln = np.arange(sizes[c], dtype=np.int64)
        pool = np.zeros((128, NW * GPC), dtype=ml_dtypes.bfloat16)
        g_of = batch[bounds[c] + ln].astype(np.int64) - c * GPC
        pool[ln % 128, (ln // 128) * GPC + g_of] = 1.0

        m = {"gidx": gidx, "ohcache": ohc, "pool": pool}
        m.update(shared)
        in_maps.append(m)
    return in_maps


def _build_program(stage=99):
    nc = bacc.Bacc("TRN2", target_bir_lowering=False, debug=False,
                   num_devices=NC, num_swdge_queues=NQ)
    AF = mybir.ActivationFunctionType
    OP = mybir.AluOpType

    ap = {}
    big_inputs = ([] if stage == 0 else [
        ("xtab", [TAB, F], bf16),
        ("gidx", [128, GIDXCOLS], i16),
        ("ohcache", [128, NGROUPS * 128], fp8),
        ("pool", [128, NW * GPC], bf16),
    ])
    for name, shape, dt in big_inputs + [
        ("identf", [128, 128], f32), ("identb", [128, 128], bf16),
        ("convw", [F, 6 * F], bf16), ("convb", [F, 6], f32),
        ("jkw", [F, 6 * F], bf16), ("jkb", [F, NB], f32),
        ("bns", [F, NB], f32), ("bnt", [F, NB], f32),
        ("l1w", [F, NB * F], f32), ("l1b", [F, 1], f32),
        ("l2w", [F, 10], f32), ("l2b", [10, 1], f32),
    ]:
        ap[name] = nc.dram_tensor(name, shape, dt, kind="ExternalInput").ap()
    out_ap = nc.dram_tensor("out", [GPC, 10], f32, kind="ExternalOutput").ap()

    with tile.TileContext(nc) as tc:
        with (
            tc.tile_pool(name="dram", bufs=1, space="DRAM") as dram,
            tc.tile_pool(name="pers", bufs=1) as pers,
            tc.tile_pool(name="rot", bufs=1) as rot,
            tc.tile_pool(name="psum", bufs=1, space="PSUM") as psum,
        ):
            ag_in = dram.tile([S, F], bf16)
            ag_const = dram.tile([S, F], bf16)

            # ---- persistent SBUF loads
            sb = {}
            for name in ((["gidx", "pool"] if stage > 0 else []) +
                         ["identf", "identb", "convw", "convb",
                          "jkw", "jkb", "bns", "bnt", "l1w", "l1b", "l2w",
                          "l2b"]):
                t_ = pers.tile(list(ap[name].shape), ap[name].dtype,
                               name=f"sb_{name}")
                nc.sync.dma_start(t_[:], ap[name][:])
                sb[name] = t_

            h1_fm = pers.tile([128, S], bf16, name="h1_fm")
            if SKIP_FIN:
                nc.scalar.copy(h1_fm[:, 0:128], sb["identb"][:])
            h2_fm = pers.tile([128, S], bf16, name="h2_fm")
            hb_fm = pers.tile([128, S], bf16, name="hb_fm")
            part = pers.tile([128, NW, 128], bf16, name="part")
            z_sb = pers.tile([128, NB, GPC], f32, name="z_sb")

            qctr = [0]
            _cset = set(CACHED_UNITS)
            if stage == 0:
                ohsb = None
            if AG_DECOUPLE:
                for _w in range(NW):
                    nc.sync.dma_start(ag_const[_w * 128:(_w + 1) * 128, :],
                                      sb["identb"][:])
            if stage > 0:
                Gpre = pers.tile([128, MAXG, F], bf16, name="Gpre")
                nc.sync.dma_start(Gpre[:, 0:1, :], ap["xtab"][0:128, :])
                OHpre = pers.tile([128, MAXG, 128], fp8, name="OHpre")
                nc.sync.dma_start(OHpre[:, 0:1, :], ap["ohcache"][:, 0:128])
            if CGROUPS and stage > 0:
                ohsb = pers.tile([128, CGROUPS, 128], fp8, name="ohsb")
                nc.sync.dma_start(ohsb[:], ap["ohcache"][:, 0:CGROUPS * 128])
            else:
                ohsb = None

            def unit_tiles(tables, k, wp):
                u = (k, wp)
                ng = UNIT_NIDX[u] // 128
                if SKIP_G:
                    G = Gpre
                else:
                    from contextlib import nullcontext
                    G = rot.tile([128, MAXG, F], bf16, tag="G", bufs=GB, name="G")
                    with (tc.high_priority(offset=GPRI) if GPRI
                          else nullcontext()):
                        nc.gpsimd.dma_gather(
                            out_ap=G[:, 0:ng, :], in_ap=tables[k][:],
                            idxs_ap=sb["gidx"][:, UNIT_GCOL[u]:
                                               UNIT_GCOL[u] + UNIT_NIDX[u] // 16],
                            num_idxs=UNIT_NIDX[u], num_idxs_reg=UNIT_NIDX[u],
                            elem_size=F, single_packet=SP1,
                            queue_num=qctr[0] % GQN)
                    qctr[0] += 1
                if SKIP_OH:
                    OH, ob = OHpre, 0
                elif u in _cset:
                    OH, ob = ohsb, OHGB[u]
                else:
                    from contextlib import nullcontext
                    OH = rot.tile([128, MAXG, 128], fp8, tag="OH", bufs=OHB,
                                  name="OH")
                    src_ap = ap["ohcache"][:, OHGB[u] * 128:
                                           (OHGB[u] + ng) * 128]
                    with (tc.high_priority(offset=OHPRI) if OHPRI
                          else nullcontext()):
                        if OH_ENG == "gps3":
                            nc.gpsimd.dma_start(OH[:, 0:ng, :], src_ap,
                                                queue_num=NQ - 1)
                        elif OH_ENG == "act":
                            nc.scalar.dma_start(OH[:, 0:ng, :], src_ap)
                        else:
                            nc.sync.dma_start(OH[:, 0:ng, :], src_ap)
                    ob = 0
                return G, OH, ob

            def ag_chunk(i, k):
                src = ag_const if AG_DECOUPLE else ag_in
                tk = dram.tile([CHROWS, F], bf16, addr_space="Shared",
                               tag=f"t{_rep[0]}_{i}_{k}",
                               name=f"t{_rep[0]}_{i}_{k}")
                nc.gpsimd.collective_compute(
                    "AllGather", OP.bypass,
                    replica_groups=[list(range(NC))],
                    ins=[src[k * CHS:(k + 1) * CHS, :].opt()],
                    outs=[tk.opt()])
                # gathers from Shared-space DRAM run ~60% slower than from
                # regular DRAM; copy the table out (split across both HWDGE
                # rings to halve the serial latency) before gathering from it
                tl = dram.tile([CHROWS, F], bf16,
                               tag=f"l{_rep[0]}_{i}_{k}",
                               name=f"l{_rep[0]}_{i}_{k}")
                h = CHROWS // 2
                nc.sync.dma_start(tl[0:h, :], tk[0:h, :])
                nc.scalar.dma_start(tl[h:, :], tk[h:, :])
                return tl

            def ag_full(i):
                # single collective for the whole shard (core-major output),
                # reshuffled to the 2 chunk-major tables during the copy-out
                src = ag_const if AG_DECOUPLE else ag_in
                tk = dram.tile([NC * S, F], bf16, addr_space="Shared",
                               tag=f"t{_rep[0]}_{i}", name=f"t{_rep[0]}_{i}")
                nc.gpsimd.collective_compute(
                    "AllGather", OP.bypass,
                    replica_groups=[list(range(NC))],
                    ins=[src[:].opt()], outs=[tk.opt()])
                tabs = []
                for k in range(NCH):
                    tl = dram.tile([CHROWS, F], bf16,
                                   tag=f"l{_rep[0]}_{i}_{k}",
                                   name=f"l{_rep[0]}_{i}_{k}")
                    for co in range(NC):
                        eng = nc.sync if co % 2 == 0 else nc.scalar
                        eng.dma_start(
                            tl[co * CHS:(co + 1) * CHS, :],
                            tk[co * S + k * CHS:co * S + (k + 1) * CHS, :])
                    tabs.append(tl)
                return tabs

            def conv(lk, tables, h_out, ag, jkli=None):
                if jkli is not None:
                    pooled = psum.tile([128, GPC], f32, tag="pooled", bufs=1,
                                       name="pooled")
                # chunk-0 pass: accumulate in PSUM, evict partials to SBUF
                for wp in range(NWP):
                    G, OH, ob = unit_tiles(tables, 0, wp)
                    cha, chb = UNIT_CH[(0, wp)]
                    for half, ch in ((0, cha), (1, chb)):
                        w = 2 * wp + half
                        gb = half * cha
                        pp = psum.tile([128, 128], f32, tag="pp", bufs=PPB,
                                       name="pp")
                        for c in range(ch):
                            nc.tensor.matmul(pp[:], G[:, gb + c, :],
                                             OH[:, ob + gb + c, :],
                                             start=(c == 0), stop=(c == ch - 1))
                        if not SKIP_PART:
                            nc.scalar.copy(part[:, w, :], pp[:])
                # chunk-1 pass: re-inject partial, finish, apply W+bias+ReLU
                tabs = []
                for wp in range(NWP):
                    G, OH, ob = unit_tiles(tables, 1, wp)
                    cha, chb = UNIT_CH[(1, wp)]
                    for half, ch in ((0, cha), (1, chb)):
                        w = 2 * wp + half
                        gb = half * cha
                        pp = psum.tile([128, 128], f32, tag="pp", bufs=PPB,
                                       name="pp")
                        if SKIP_PART:
                            nc.tensor.matmul(pp[:], sb["identb"][:],
                                             sb["identb"][:],
                                             start=True, stop=False)
                        else:
                            nc.tensor.matmul(pp[:], sb["identb"][:],
                                             part[:, w, :],
                                             start=True, stop=False)
                        for c in range(ch):
                            nc.tensor.matmul(pp[:], G[:, gb + c, :],
                                             OH[:, ob + gb + c, :],
                                             start=False, stop=(c == ch - 1))
                        if SKIP_FIN:
                            continue
                        aggs = rot.tile([128, 128], bf16, tag="aggs", bufs=4,
                                        name="aggs")
                        nc.scalar.copy(aggs[:], pp[:])
                        hn = psum.tile([128, 128], f32, tag="hn", bufs=2,
                                       name="hn")
                        nc.tensor.matmul(hn[:], sb["convw"][:, lk * F:(lk + 1) * F],
                                         aggs[:], start=True, stop=True)
                        nc.scalar.activation(h_out[:, w * 128:(w + 1) * 128],
                                             hn[:], AF.Relu,
                                             bias=sb["convb"][:, lk:lk + 1])
                        if jkli is not None:
                            # fused JumpingKnowledge: hb = relu(jk_w @ [h1;h2])
                            li = jkli
                            hb = psum.tile([128, 128], f32, tag="hn", bufs=2,
                                           name="hb")
                            nc.tensor.matmul(
                                hb[:],
                                sb["jkw"][:, (2 * li) * F:(2 * li + 1) * F],
                                h1_fm[:, w * 128:(w + 1) * 128],
                                start=True, stop=False)
                            nc.tensor.matmul(
                                hb[:],
                                sb["jkw"][:, (2 * li + 1) * F:(2 * li + 2) * F],
                                h_out[:, w * 128:(w + 1) * 128],
                                start=False, stop=True)
                            nc.scalar.activation(hb_fm[:, w * 128:(w + 1) * 128],
                                                 hb[:], AF.Relu,
                                                 bias=sb["jkb"][:, li:li + 1])
                            src_fm = hb_fm
                        else:
                            src_fm = h_out
                        if ag is not None or jkli is not None:
                            hnT = psum.tile([128, 128], bf16, tag="hnT",
                                            bufs=1, name="hnT")
                            nc.tensor.transpose(hnT[:],
                                                src_fm[:, w * 128:(w + 1) * 128],
                                                sb["identb"][:])
                            hcol = rot.tile([128, 128], bf16, tag="hcol",
                                            bufs=4, name="hcol")
                            nc.scalar.copy(hcol[:], hnT[:])
                        if jkli is not None:
                            nc.tensor.matmul(pooled[:], hcol[:],
                                             sb["pool"][:, w * GPC:(w + 1) * GPC],
                                             start=(w == 0), stop=(w == NW - 1))
                        if ag is not None:
                            nc.sync.dma_start(ag_in[w * 128:(w + 1) * 128, :],
                                              hcol[:])
                            # trigger the chunk AG as soon as its input rows
                            # are all written (w24 -> chunk0, w49 -> chunk1)
                            if AGMODE == 2:
                                if w == NW // 2 - 1:
                                    tabs.append(ag_chunk(ag, 0))
                                elif w == NW - 1:
                                    tabs.append(ag_chunk(ag, 1))
                            elif w == NW - 1:
                                tabs.extend(ag_full(ag))
                if jkli is not None:
                    nc.scalar.copy(z_sb[:, jkli, :], pooled[:])
                return tabs if ag is not None else None

            def jk(li, ag):
                last = ag is None
                tabs = []
                pooled = psum.tile([128, GPC], f32, tag="pooled", bufs=1,
                                   name="pooled")
                for w in range(NW):
                    hb = psum.tile([128, 128], f32, tag="hn", bufs=2,
                                   name="hb")
                    nc.tensor.matmul(hb[:],
                                     sb["jkw"][:, (2 * li) * F:(2 * li + 1) * F],
                                     h1_fm[:, w * 128:(w + 1) * 128],
                                     start=True, stop=False)
                    nc.tensor.matmul(hb[:],
                                     sb["jkw"][:, (2 * li + 1) * F:(2 * li + 2) * F],
                                     h2_fm[:, w * 128:(w + 1) * 128],
                                     start=False, stop=True)
                    nc.scalar.activation(hb_fm[:, w * 128:(w + 1) * 128],
                                         hb[:], AF.Relu,
                                         bias=sb["jkb"][:, li:li + 1])
                    hnT = psum.tile([128, 128], bf16, tag="hnT", bufs=1,
                                    name="hnT")
                    nc.tensor.transpose(hnT[:], hb_fm[:, w * 128:(w + 1) * 128],
                                        sb["identb"][:])
                    hcol = rot.tile([128, 128], bf16, tag="hcol", bufs=4,
                                    name="hcol")
                    nc.scalar.copy(hcol[:], hnT[:])
                    if not last:
                        nc.sync.dma_start(ag_in[w * 128:(w + 1) * 128, :],
                                          hcol[:])
                        if AGMODE == 2:
                            if w == NW // 2 - 1:
                                tabs.append(ag_chunk(ag, 0))
                            elif w == NW - 1:
                                tabs.append(ag_chunk(ag, 1))
                        elif w == NW - 1:
                            tabs.extend(ag_full(ag))
                    nc.tensor.matmul(pooled[:], hcol[:],
                                     sb["pool"][:, w * GPC:(w + 1) * GPC],
                                     start=(w == 0), stop=(w == NW - 1))
                nc.scalar.copy(z_sb[:, li, :], pooled[:])
                return tabs if not last else None

            # ---- main flow
            steps = [
                lambda: conv(0, [ap["xtab"][k * CHROWS:(k + 1) * CHROWS, :]
                                 for k in range(NCH)], h1_fm, 0),
                lambda t: conv(1, t, h2_fm, 1, jkli=0),
                lambda t: conv(2, t, h1_fm, 2),
                lambda t: conv(3, t, h2_fm, 3, jkli=1),
                lambda t: conv(4, t, h1_fm, 4),
                lambda t: conv(5, t, h2_fm, None, jkli=2),
            ]
            _rep = [0]
            for rep in range(REPEAT):
                _rep[0] = rep
                table = None
                for i, st in enumerate(steps):
                    if i >= stage:
                        break
                    r = st(table) if st.__code__.co_argcount else st()
                    if r is not None:
                        table = r

            # ---- head
            if stage < 6:
                outt0 = rot.tile([GPC, 10], f32, tag="outt", bufs=1,
                                 name="outt0")
                nc.vector.tensor_copy(outt0[:], h1_fm[0:GPC, 0:10])
                nc.sync.dma_start(out_ap[:], outt0[:])
            else:
                _head(nc, tc, rot, psum, sb, z_sb, out_ap)

    nc.compile()
    return nc


def _head(nc, tc, rot, psum, sb, z_sb, out_ap):
    AF = mybir.ActivationFunctionType
    OP = mybir.AluOpType
    zbn = rot.tile([128, NB, GPC], f32, tag="zbn", bufs=1, name="zbn")
    for t in range(NB):
        nc.vector.tensor_scalar(
            out=zbn[:, t, :], in0=z_sb[:, t, :],
            scalar1=sb["bns"][:, t:t + 1], scalar2=sb["bnt"][:, t:t + 1],
            op0=OP.mult, op1=OP.add)
    a1 = psum.tile([128, GPC], f32, tag="hn", bufs=2, name="a1")
    for t in range(NB):
        nc.tensor.matmul(a1[:], sb["l1w"][:, t * F:(t + 1) * F],
                         zbn[:, t, :], start=(t == 0), stop=(t == NB - 1))
    a1s = rot.tile([128, GPC], f32, tag="a1s", bufs=1, name="a1s")
    nc.scalar.activation(a1s[:], a1[:], AF.Relu, bias=sb["l1b"][:])
    z2 = psum.tile([10, GPC], f32, tag="pooled", bufs=1, name="z2")
    nc.tensor.matmul(z2[:], sb["l2w"][:], a1s[:], start=True, stop=True)
    z2s = rot.tile([10, GPC], f32, tag="z2s", bufs=1, name="z2s")
    nc.scalar.activation(z2s[:], z2[:], AF.Identity, bias=sb["l2b"][:])
    z2T = psum.tile([GPC, 10], f32, tag="hnT", bufs=1, name="z2T")
    nc.tensor.transpose(z2T[:], z2s[:], sb["identf"][0:10, 0:10])
    z2Ts = rot.tile([GPC, 10], f32, tag="z2Ts", bufs=1, name="z2Ts")
    nc.vector.tensor_copy(z2Ts[:], z2T[:])
    negm = rot.tile([GPC, 1], f32, tag="negm", bufs=1, name="negm")
    nc.vector.tensor_reduce(negm[:], z2Ts[:], mybir.AxisListType.X,
                            OP.max, negate=True)
    et = rot.tile([GPC, 10], f32, tag="et", bufs=1, name="et")
    nc.scalar.activation(et[:], z2Ts[:], AF.Exp, bias=negm[:])
    ssum = rot.tile([GPC, 1], f32, tag="ssum", bufs=1, name="ssum")
    nc.vector.tensor_reduce(ssum[:], et[:], mybir.AxisListType.X, OP.add)
    rcp = rot.tile([GPC, 1], f32, tag="rcp", bufs=1, name="rcp")
    nc.vector.reciprocal(rcp[:], ssum[:])
    outt = rot.tile([GPC, 10], f32, tag="outt", bufs=1, name="outt")
    nc.vector.tensor_scalar_mul(outt[:], et[:], rcp[:])
    nc.sync.dma_start(out_ap[:], outt[:])


def _get_program():
    global _PROGRAM
    if _PROGRAM is None:
        _PROGRAM = _build_program()
    return _PROGRAM


def kernel(**inputs) -> np.ndarray:
    in_maps = _preprocess(inputs)
    nc = _get_program()
    res = run_bass_kernel_spmd(nc, in_maps, list(range(NC)))
    return np.concatenate([res.results[c]["out"] for c in range(NC)], axis=0)
